# revision 27
# baseline (speedup 1.0000x reference)
"""Trainium2 Bass kernel for nn_CausalSelfAttention_52905407152466.

BitNet-style causal self-attention, distributed over 8 NeuronCores:
  - token-sharded QKV projections (512 tokens/core, full weights/core)
  - AllToAll #1 -> head-sharded attention (2 heads x B=2 per core)
  - AllToAll #2 -> token-sharded output projection

Numeric strategy: ternary weights are exact in fp16, so all projection
matmuls run in fp16 losslessly given fp16 activations. Attention runs in
fp16 (inputs ~2^-11 rounded, fp32 accumulation). The softmax skips the
max-subtraction (scores are bounded ~4) so exp folds into one activation
instruction per score group; the normalizer Z comes from an ones-column
appended to V. Causal masking = gpsimd affine_select on the exp output of
diagonal tiles. The final Wo projection uses the exact int8 path
(int8 x ternary in fp16 = exact integer accumulation in fp32).
"""

import numpy as np

import concourse.bacc as bacc
import concourse.mybir as mybir
import concourse.tile as tile
from concourse.bass_utils import run_bass_kernel_spmd
from concourse.masks import make_identity

F32 = mybir.dt.float32
F16 = mybir.dt.float16
I8 = mybir.dt.int8
AX = mybir.AxisListType
OP = mybir.AluOpType
ACTF = mybir.ActivationFunctionType

NCORES = 8
B, T, C = 2, 2048, 1024
H, D = 16, 64
BT = B * T                  # 4096 flat tokens
TPC = BT // NCORES          # 512 tokens per core
HPC = H // NCORES           # 2 heads per core
NT = TPC // 128             # 4 token tiles per core
NCT = C // 128              # 8 channel tiles
QB = 512                    # query block (free dim of score matmuls)
NQB = T // QB               # 4 query blocks per (b, h) instance
KT = 128                    # key tile (partition dim of scores)
ROPE_BASE = 10000.0

_CACHE = {}


def _host_tables(core):
    """Per-core RoPE tables in [128 = 2 interleaved heads x (32 lo | 32 hi), TPC] f16."""
    pos0 = (core * TPC) % T
    pos = np.arange(pos0, pos0 + TPC, dtype=np.float64)
    inv = 1.0 / (ROPE_BASE ** (np.arange(0, D, 2, dtype=np.float64) / D))
    ang = pos[None, :] * inv[:, None]              # [32, TPC]
    cos = np.cos(ang).astype(np.float32).astype(np.float16)
    sin = np.sin(ang).astype(np.float32).astype(np.float16)
    # rope as q*cos + (J q)*sin with J the half-swap sign matrix
    t1 = np.concatenate([cos, cos, cos, cos], axis=0)
    t2 = np.concatenate([sin, sin, sin, sin], axis=0)
    return t1.astype(np.float16), t2.astype(np.float16)


def _host_jt():
    i32 = np.eye(32, dtype=np.float16)
    z = np.zeros((32, 32), np.float16)
    j64 = np.block([[z, -i32], [i32, z]])     # J: Jq[0:32] = -q[32:64]; Jq[32:64] = q[0:32]
    jt = np.block([[j64.T, np.zeros((64, 64), np.float16)],
                   [np.zeros((64, 64), np.float16), j64.T]])
    return jt.astype(np.float16)


def build_program():
    nc = bacc.Bacc("TRN2", target_bir_lowering=False, debug=False,
                   num_devices=NCORES)
    io = {}

    def inp(name, shape, dtype=F32):
        io[name] = nc.declare_dram_parameter(name, list(shape), dtype, isOutput=False)
        return io[name]

    def outp(name, shape, dtype=F32):
        io[name] = nc.declare_dram_parameter(name, list(shape), dtype, isOutput=True)
        return io[name]

    x_d = inp("x_slice", (TPC, C))
    w_d = {n: inp(n + "T", (C, C)) for n in ("Wq", "Wk", "Wv", "Wo")}
    t1_d = inp("ropeT1", (128, TPC), F16)
    t2_d = inp("ropeT2", (128, TPC), F16)
    jt_d = inp("ropeJT", (128, 128), F16)
    out_d = outp("out_slice", (TPC, C))

    import os
    skip_coll = os.environ.get("SKIP_COLL", "0") == "1"
    # layout per shard: q [128, TPC], k [128, TPC], v [128, NT, 130]
    with tile.TileContext(nc) as tc:
        with tc.tile_pool(name="dram", bufs=1, space="DRAM") as dram:
            a2a1_in = dram.tile([NCORES, 2 * 128 * TPC], F16)
            a2a1_out = dram.tile([NCORES, 2 * 128 * TPC], F16)
            a2av_in = dram.tile([NCORES, 128 * NT * 130], F16)
            a2av_out = dram.tile([NCORES, 128 * NT * 130], F16)
            a2a2_in = dram.tile([NCORES, 128 * NT * 2 * D], F16)
            a2a2_out = dram.tile([NCORES, 128 * NT * 2 * D], F16)

            _build_body(nc, tc, io, a2a1_in, a2a1_out, a2av_in, a2av_out,
                        a2a2_in, a2a2_out, skip_coll=skip_coll)
    nc.compile()
    return nc


def _build_body(nc, tc, io, a2a1_in, a2a1_out, a2av_in, a2av_out,
                a2a2_in, a2a2_out, skip_coll=False):
    from contextlib import ExitStack
    es = ExitStack()
    ident_pool = es.enter_context(tc.tile_pool(name="const", bufs=1))
    sb = es.enter_context(tc.tile_pool(name="sb", bufs=1))
    wl = es.enter_context(tc.tile_pool(name="wl", bufs=2))
    esp = ExitStack()
    ps = esp.enter_context(tc.tile_pool(name="mmps", bufs=3, space="PSUM"))
    psy = esp.enter_context(tc.tile_pool(name="psy", bufs=1, space="PSUM"))

    # ---------------- constants -------------------------------------------
    ident = ident_pool.tile([128, 128], F16)
    make_identity(nc, ident[:])
    t1 = ident_pool.tile([128, TPC], F16)
    t2 = ident_pool.tile([128, TPC], F16)
    nc.sync.dma_start(t1[:], io["ropeT1"][:])
    nc.sync.dma_start(t2[:], io["ropeT2"][:])
    jt = ident_pool.tile([128, 128], F16)
    nc.sync.dma_start(jt[:], io["ropeJT"][:])
    ones_col = ident_pool.tile([128, 1], F16)
    nc.gpsimd.memset(ones_col[:], 1.0)

    # ---------------- P0: x load + act_quant + transpose ------------------
    xsb = sb.tile([128, NT, C], F32)
    nc.sync.dma_start(xsb[:], io["x_slice"].rearrange("(n p) c -> p n c", p=128))
    xq16 = sb.tile([128, NT, C], F16)
    for tt in range(NT):
        mx = sb.tile([128, 1], F32, tag="mx")
        nc.vector.tensor_reduce(mx[:], xsb[:, tt], axis=AX.X, op=OP.max,
                                apply_absolute_value=True)
        sc = sb.tile([128, 1], F32, tag="sc")   # 1/st = clip(mx)/127
        nc.vector.tensor_scalar(sc[:], mx[:], 1e-5, 1.0 / 127.0,
                                op0=OP.max, op1=OP.mult)
        st = sb.tile([128, 1], F32, tag="st")   # 127/clip(mx)
        nc.vector.reciprocal(st[:], sc[:])
        xq8 = sb.tile([128, C], I8, tag="xq8")
        nc.vector.tensor_scalar(xq8[:], xsb[:, tt], st[:], None, op0=OP.mult)
        nc.vector.tensor_scalar(xq16[:, tt], xq8[:], sc[:], None, op0=OP.mult)
    # transpose -> xqT [c, t] tiles (PE transpose, psum bounce)
    xqT = sb.tile([128, NCT, TPC], F16)
    for ct in range(NCT):
        for tt in range(NT):
            trx = psy.tile([128, 128], F16, tag="trx")
            nc.tensor.transpose(trx[:], xq16[:, tt, 128 * ct:128 * (ct + 1)], ident[:])
            nc.scalar.activation(xqT[:, ct, 128 * tt:128 * (tt + 1)], trx[:], ACTF.Copy)

    # ---------------- weights helper ---------------------------------------
    wT = {}
    swcol = {}
    ones128 = ident_pool.tile([1, 128], F32)
    nc.gpsimd.memset(ones128[:], 1.0)
    onescol32 = ident_pool.tile([128, 1], F32)
    nc.gpsimd.memset(onescol32[:], 1.0)

    def prep_weight(wn):
        wsb = wl.tile([128, NCT, C], F32, tag="wload", name=f"wload_{wn}")
        nc.sync.dma_start(wsb[:], io[wn + "T"].rearrange("(n p) c -> p n c", p=128))
        asums = sb.tile([128, NCT], F32, tag="asums", name=f"asums_{wn}")
        for ot in range(NCT):
            nc.vector.tensor_reduce(asums[:, ot:ot + 1], wsb[:, ot], axis=AX.X,
                                    op=OP.add, apply_absolute_value=True)
        atot = sb.tile([128, 1], F32, tag="atot", name=f"atot_{wn}")
        nc.vector.tensor_reduce(atot[:], asums[:], axis=AX.X, op=OP.add)
        sw_ps = psy.tile([1, 1], F32, tag="swps", name=f"swps_{wn}")
        nc.tensor.matmul(sw_ps[:], onescol32[:], atot[:], start=True, stop=True)
        sw = sb.tile([1, 1], F32, tag="sw", name=f"sw_{wn}")
        nc.vector.tensor_scalar(sw[:], sw_ps[:], 1.0 / (C * C), 1e-5,
                                op0=OP.mult, op1=OP.max)
        swb_ps = psy.tile([128, 1], F32, tag="swbps", name=f"swbps_{wn}")
        nc.tensor.matmul(swb_ps[:], ones128[:], sw[:], start=True, stop=True)
        swc = sb.tile([128, 1], F32, tag=f"swc_{wn}", name=f"swc_{wn}")
        nc.vector.tensor_copy(swc[:], swb_ps[:])
        swcol[wn] = swc
        inv_s = sb.tile([128, 1], F32, tag="inv_s", name=f"invs_{wn}")
        nc.vector.reciprocal(inv_s[:], swc[:])
        wtag = "wTs_0" if wn in ("Wq", "Wv") else "wTs_1"
        wTt = sb.tile([128, NCT, C], F16, tag=wtag, name=f"wT_{wn}")
        for ot in range(NCT):
            w8 = sb.tile([128, C], I8, tag="w8", name=f"w8_{wn}{ot}", bufs=2)
            nc.vector.tensor_scalar(w8[:], wsb[:, ot], inv_s[:], None, op0=OP.mult)
            nc.vector.tensor_scalar(wTt[:, ot], w8[:], 1, -1,
                                    op0=OP.min, op1=OP.max)
        wT[wn] = wTt

    def proj_qk(name, dst):
        for ot in range(NCT):
            mm_ps = ps.tile([128, TPC], F32, tag="mm512", name=f"mmps_{name}{ot}")
            for ct in range(NCT):
                nc.tensor.matmul(mm_ps[:], wT[name][:, ct, 128 * ot:128 * (ot + 1)],
                                 xqT[:, ct], start=(ct == 0), stop=(ct == NCT - 1))
            raw = sb.tile([128, TPC], F16, tag="qkraw", name=f"raw_{name}{ot}")
            nc.vector.tensor_copy(raw[:], mm_ps[:])
            jq_ps = ps.tile([128, TPC], F32, tag="mm512", name=f"jq_{name}{ot}")
            nc.tensor.matmul(jq_ps[:], jt[:], raw[:], start=True, stop=True)
            p1 = sb.tile([128, TPC], F16, tag="ropep1", name=f"p1_{name}{ot}")
            p2 = sb.tile([128, TPC], F16, tag="ropep2", name=f"p2_{name}{ot}")
            nc.vector.tensor_tensor(p1[:], raw[:], t1[:], op=OP.mult)
            nc.vector.tensor_tensor(p2[:], jq_ps[:], t2[:], op=OP.mult)
            nc.vector.tensor_tensor(dst[:, ot], p1[:], p2[:], op=OP.add)

    QSZ = 128 * TPC
    VSZ = 128 * NT * 130

    # q/k pipeline -> atoa-qk as early as possible
    qTr = sb.tile([128, NCT, TPC], F16)
    kTr = sb.tile([128, NCT, TPC], F16)
    prep_weight("Wq")
    proj_qk("Wq", qTr)
    nc.sync.dma_start(a2a1_in[:, 0:QSZ].rearrange("d (p t) -> p d t", p=128), qTr[:])
    prep_weight("Wk")
    proj_qk("Wk", kTr)
    nc.sync.dma_start(a2a1_in[:, QSZ:2 * QSZ].rearrange("d (p t) -> p d t", p=128),
                      kTr[:])
    if skip_coll:
        nc.sync.dma_start(a2a1_out[:], a2a1_in[:])
    else:
        nc.gpsimd.collective_compute(
            "AllToAll", OP.bypass, replica_groups=[list(range(NCORES))],
            ins=[a2a1_in.opt()], outs=[a2a1_out.opt()])

    qTa = sb.tile([128, BT], F16, tag="qTr")     # reuse qTr slot (dead after send)
    kTa = sb.tile([128, BT], F16, tag="kTr")
    va = sb.tile([128, BT // 128, 2, 65], F16, tag="v_sb")   # reuse v_sb slot
    nc.scalar.dma_start(qTa[:].rearrange("p (s t) -> p s t", s=NCORES),
                        a2a1_out[:, 0:QSZ].rearrange("s (p t) -> p s t", p=128))
    nc.scalar.dma_start(kTa[:].rearrange("p (s t) -> p s t", s=NCORES),
                        a2a1_out[:, QSZ:2 * QSZ].rearrange("s (p t) -> p s t", p=128))

    # exp scale column: swq*swk/8 -> [128,1] f32
    expsc = sb.tile([128, 1], F32)
    nc.vector.tensor_tensor(expsc[:], swcol["Wq"][:], swcol["Wk"][:], op=OP.mult)
    nc.vector.tensor_scalar(expsc[:], expsc[:], 1.0 / np.sqrt(np.float64(D)), None,
                            op0=OP.mult)

    # v pipeline -> atoa-v
    prep_weight("Wv")
    v_sb = sb.tile([128, NT, H, 65], F16)
    nc.gpsimd.memset(v_sb[:], 1.0)
    for tt in range(NT):
        for ob in range(2):
            mm_ps = ps.tile([128, 512], F32, tag="mm512", name=f"vps_{tt}{ob}")
            for ct in range(NCT):
                nc.tensor.matmul(mm_ps[:], xqT[:, ct, 128 * tt:128 * (tt + 1)],
                                 wT["Wv"][:, ct, 512 * ob:512 * (ob + 1)],
                                 start=(ct == 0), stop=(ct == NCT - 1))
            nc.scalar.activation(
                v_sb[:, tt, 8 * ob:8 * (ob + 1), 0:64], mm_ps[:],
                ACTF.Copy, scale=swcol["Wv"][:])
    for dst in range(NCORES):
        nc.sync.dma_start(
            a2av_in[dst].rearrange("(p n v) -> p n v", p=128, n=NT),
            v_sb[:, :, 2 * dst:2 * dst + 2, :].rearrange("p n h v -> p n (h v)"))
    if skip_coll:
        nc.sync.dma_start(a2av_out[:], a2av_in[:])
    else:
        nc.gpsimd.collective_compute(
            "AllToAll", OP.bypass, replica_groups=[list(range(NCORES))],
            ins=[a2av_in.opt()], outs=[a2av_out.opt()])

    for s in range(NCORES):
        nc.sync.dma_start(
            va[:, NT * s:NT * (s + 1)].rearrange("p n h v -> p n (h v)"),
            a2av_out[s].rearrange("(p n v) -> p n v", p=128, n=NT))

    # Wo prep overlaps the collectives / attention start
    prep_weight("Wo")

    # ---------------- P4: attention ----------------------------------------
    # per (head, batch, q-block): scores (transposed) -> exp -> mask -> AV
    esp.close()
    y_sb = sb.tile([128, BT // 128, 2, D], F16)   # [qt-part, qt-tile, head, d]
    exp_sb_pool = es.enter_context(tc.tile_pool(name="expp", bufs=3))
    esp = ExitStack()
    score_ps_pool = esp.enter_context(tc.tile_pool(name="scps", bufs=2, space="PSUM"))
    yaug_ps_pool = esp.enter_context(tc.tile_pool(name="yaug", bufs=2, space="PSUM"))
    tr_ps_pool = esp.enter_context(tc.tile_pool(name="trps", bufs=2, space="PSUM"))
    KG = 2          # k-tiles per exp group (psum banks per score group)
    for b in range(B):
        base = b * T
        for jb in range(NQB):
            qs = base + QB * jb           # q-block col offset
            for h in range(HPC):
                yaug = yaug_ps_pool.tile([65, QB], F32, tag="yaug")
                nkt = (jb + 1) * (QB // KT)       # causal k-tiles for this block
                for kg in range(nkt // KG):
                    sgrp = score_ps_pool.tile([128, KG * QB], F32, tag="sgrp")
                    for m in range(KG):
                        kt_i = kg * KG + m
                        ks = base + KT * kt_i
                        nc.tensor.matmul(
                            sgrp[:, QB * m:QB * (m + 1)],
                            kTa[64 * h:64 * (h + 1), ks:ks + KT],
                            qTa[64 * h:64 * (h + 1), qs:qs + QB],
                            start=True, stop=True,
                            tile_position=(64 * h, 0))
                    egrp = exp_sb_pool.tile([128, KG * QB], F16, tag=f"egrp{h}")
                    nc.scalar.activation(egrp[:], sgrp[:], ACTF.Exp, scale=expsc[:])
                    for m in range(KG):
                        kt_i = kg * KG + m
                        mbase = QB * jb - KT * kt_i
                        if mbase < 127:   # diagonal tile: causal mask needed
                            nc.gpsimd.affine_select(
                                out=egrp[:, QB * m:QB * (m + 1)],
                                in_=egrp[:, QB * m:QB * (m + 1)],
                                compare_op=OP.is_ge, fill=0.0,
                                base=mbase, pattern=[[1, QB]],
                                channel_multiplier=-1)
                    for m in range(KG):
                        kt_i = kg * KG + m
                        gt = base // 128 + kt_i
                        nc.tensor.matmul(yaug[:], va[:, gt, h, :],
                                         egrp[:, QB * m:QB * (m + 1)],
                                         start=(kg == 0 and m == 0),
                                         stop=(kg == nkt // KG - 1 and m == KG - 1))
                # epilogue: copy, transpose 128-chunks, normalize
                yaug16 = exp_sb_pool.tile([65, QB], F16, tag=f"yaug16_{h}")
                nc.vector.tensor_copy(yaug16[:], yaug[:])
                for ch in range(QB // 128):
                    trp = tr_ps_pool.tile([128, 65], F16, tag="trp")
                    nc.tensor.transpose(trp[:], yaug16[:, 128 * ch:128 * (ch + 1)],
                                        ident[0:65, 0:65])
                    rec = exp_sb_pool.tile([128, 1], F32, tag=f"rec{h}")
                    nc.vector.reciprocal(rec[:], trp[:, 64:65])
                    nc.vector.tensor_scalar(
                        y_sb[:, (qs + 128 * ch) // 128, h, :], trp[:, 0:64],
                        rec[:], None, op0=OP.mult)

    # ---------------- P5: AllToAll #2 --------------------------------------
    YSZ = 128 * NT * 2 * D
    for dst in range(NCORES):
        nc.sync.dma_start(
            a2a2_in[dst].rearrange("(p n c) -> p n c", p=128, n=NT),
            y_sb[:, NT * dst:NT * (dst + 1)].rearrange("p n h dd -> p n (h dd)"))
    if skip_coll:
        nc.sync.dma_start(a2a2_out[:], a2a2_in[:])
    else:
        nc.gpsimd.collective_compute(
            "AllToAll", OP.bypass, replica_groups=[list(range(NCORES))],
            ins=[a2a2_in.opt()], outs=[a2a2_out.opt()])
    yfull = sb.tile([128, NT, C], F16, tag="xq16")   # [t-part, t-tile, channels]
    for s in range(NCORES):
        nc.sync.dma_start(
            yfull[:, :, 128 * s:128 * (s + 1)],
            a2a2_out[s].rearrange("(p n c) -> p n c", p=128, n=NT))

    # act_quant(y) exact int8 + transpose
    esp.close()
    esp = ExitStack()
    ps = esp.enter_context(tc.tile_pool(name="ops", bufs=2, space="PSUM"))
    yq16 = sb.tile([128, NT, C], F16)
    osc = {}
    for tt in range(NT):
        mxy = sb.tile([128, 1], F32, tag="mxy")
        nc.vector.tensor_reduce(mxy[:], yfull[:, tt], axis=AX.X, op=OP.max,
                                apply_absolute_value=True)
        scy = sb.tile([128, 1], F32, tag=f"scy{tt}")
        nc.vector.tensor_scalar(scy[:], mxy[:], 1e-5, 1.0 / 127.0,
                                op0=OP.max, op1=OP.mult)
        sty = sb.tile([128, 1], F32, tag="sty")
        nc.vector.reciprocal(sty[:], scy[:])
        yq8 = sb.tile([128, C], I8, tag="yq8")
        nc.vector.tensor_scalar(yq8[:], yfull[:, tt], sty[:], None, op0=OP.mult)
        nc.vector.tensor_copy(yq16[:, tt], yq8[:])
        # output scale column: swo * scy
        oscc = sb.tile([128, 1], F32, tag=f"oscc{tt}")
        nc.vector.tensor_tensor(oscc[:], scy[:], swcol["Wo"][:], op=OP.mult)
        osc[tt] = oscc
    yqT = sb.tile([128, NCT, TPC], F16)
    for ct in range(NCT):
        for tt in range(NT):
            trx = ps.tile([128, 128], F16, tag="trx")
            nc.tensor.transpose(trx[:], yq16[:, tt, 128 * ct:128 * (ct + 1)], ident[:])
            nc.scalar.activation(yqT[:, ct, 128 * tt:128 * (tt + 1)], trx[:], ACTF.Copy)

    # ---------------- P6: Wo projection ------------------------------------
    out_sb = sb.tile([128, NT, C], F32, tag="xsb")
    for tt in range(NT):
        for ob in range(2):
            mm_ps = ps.tile([128, 512], F32, tag="mm512")
            for ct in range(NCT):
                nc.tensor.matmul(mm_ps[:], yqT[:, ct, 128 * tt:128 * (tt + 1)],
                                 wT["Wo"][:, ct, 512 * ob:512 * (ob + 1)],
                                 start=(ct == 0), stop=(ct == NCT - 1))
            nc.scalar.activation(out_sb[:, tt, 512 * ob:512 * (ob + 1)], mm_ps[:],
                                 ACTF.Copy, scale=osc[tt][:])
            nc.sync.dma_start(
                io["out_slice"].rearrange("(n p) c -> p n c", p=128)
                [:, tt, 512 * ob:512 * (ob + 1)],
                out_sb[:, tt, 512 * ob:512 * (ob + 1)])
    esp.close()
    es.close()


def kernel(x, Wq, Wk, Wv, Wo, _trace=False):
    x = np.ascontiguousarray(x, dtype=np.float32)
    if "nc" not in _CACHE:
        _CACHE["nc"] = build_program()
    nc = _CACHE["nc"]
    xf = x.reshape(BT, C)
    wqT = np.ascontiguousarray(np.asarray(Wq, np.float32).T)
    wkT = np.ascontiguousarray(np.asarray(Wk, np.float32).T)
    wvT = np.ascontiguousarray(np.asarray(Wv, np.float32).T)
    woT = np.ascontiguousarray(np.asarray(Wo, np.float32).T)
    in_maps = []
    for c in range(NCORES):
        t1, t2 = _host_tables(c)
        in_maps.append({
            "x_slice": np.ascontiguousarray(xf[TPC * c:TPC * (c + 1)]),
            "WqT": wqT, "WkT": wkT, "WvT": wvT, "WoT": woT,
            "ropeT1": t1, "ropeT2": t2, "ropeJT": _host_jt(),
        })
    res = run_bass_kernel_spmd(nc, in_maps, list(range(NCORES)), trace=_trace)
    out = np.concatenate([res.results[c]["out_slice"] for c in range(NCORES)], axis=0)
    out = out.reshape(B, T, C).astype(np.float32)
    if _trace:
        return out, res
    return out


# revision 30
# speedup vs baseline: 1.0451x; 1.0451x over previous
"""Trainium2 Bass kernel for nn_CausalSelfAttention_52905407152466.

BitNet-style causal self-attention, distributed over 8 NeuronCores:
  - token-sharded QKV projections (512 tokens/core, full weights/core)
  - AllToAll #1 -> head-sharded attention (2 heads x B=2 per core)
  - AllToAll #2 -> token-sharded output projection

Numeric strategy: ternary weights are exact in fp16, so all projection
matmuls run in fp16 losslessly given fp16 activations. Attention runs in
fp16 (inputs ~2^-11 rounded, fp32 accumulation). The softmax skips the
max-subtraction (scores are bounded ~4) so exp folds into one activation
instruction per score group; the normalizer Z comes from an ones-column
appended to V. Causal masking = gpsimd affine_select on the exp output of
diagonal tiles. The final Wo projection uses the exact int8 path
(int8 x ternary in fp16 = exact integer accumulation in fp32).
"""

import numpy as np

import concourse.bacc as bacc
import concourse.mybir as mybir
import concourse.tile as tile
from concourse.bass_utils import run_bass_kernel_spmd
from concourse.masks import make_identity

F32 = mybir.dt.float32
F16 = mybir.dt.float16
I8 = mybir.dt.int8
AX = mybir.AxisListType
OP = mybir.AluOpType
ACTF = mybir.ActivationFunctionType

NCORES = 8
B, T, C = 2, 2048, 1024
H, D = 16, 64
BT = B * T                  # 4096 flat tokens
TPC = BT // NCORES          # 512 tokens per core
HPC = H // NCORES           # 2 heads per core
NT = TPC // 128             # 4 token tiles per core
NCT = C // 128              # 8 channel tiles
QB = 512                    # query block (free dim of score matmuls)
NQB = T // QB               # 4 query blocks per (b, h) instance
KT = 128                    # key tile (partition dim of scores)
ROPE_BASE = 10000.0

_CACHE = {}


def _host_tables(core):
    """Per-core RoPE tables in [128 = 2 interleaved heads x (32 lo | 32 hi), TPC] f16."""
    pos0 = (core * TPC) % T
    pos = np.arange(pos0, pos0 + TPC, dtype=np.float64)
    inv = 1.0 / (ROPE_BASE ** (np.arange(0, D, 2, dtype=np.float64) / D))
    ang = pos[None, :] * inv[:, None]              # [32, TPC]
    cos = np.cos(ang).astype(np.float32).astype(np.float16)
    sin = np.sin(ang).astype(np.float32).astype(np.float16)
    # rope as q*cos + (J q)*sin with J the half-swap sign matrix
    t1 = np.concatenate([cos, cos, cos, cos], axis=0)
    t2 = np.concatenate([sin, sin, sin, sin], axis=0)
    return t1.astype(np.float16), t2.astype(np.float16)


def _host_jt():
    i32 = np.eye(32, dtype=np.float16)
    z = np.zeros((32, 32), np.float16)
    j64 = np.block([[z, -i32], [i32, z]])     # J: Jq[0:32] = -q[32:64]; Jq[32:64] = q[0:32]
    jt = np.block([[j64.T, np.zeros((64, 64), np.float16)],
                   [np.zeros((64, 64), np.float16), j64.T]])
    return jt.astype(np.float16)


def build_program():
    nc = bacc.Bacc("TRN2", target_bir_lowering=False, debug=False,
                   num_devices=NCORES)
    io = {}

    def inp(name, shape, dtype=F32):
        io[name] = nc.declare_dram_parameter(name, list(shape), dtype, isOutput=False)
        return io[name]

    def outp(name, shape, dtype=F32):
        io[name] = nc.declare_dram_parameter(name, list(shape), dtype, isOutput=True)
        return io[name]

    x_d = inp("x_slice", (TPC, C))
    w_d = {n: inp(n + "T", (C, C)) for n in ("Wq", "Wk", "Wv", "Wo")}
    t1_d = inp("ropeT1", (128, TPC), F16)
    t2_d = inp("ropeT2", (128, TPC), F16)
    jt_d = inp("ropeJT", (128, 128), F16)
    out_d = outp("out_slice", (TPC, C))

    import os
    skip_coll = os.environ.get("SKIP_COLL", "0") == "1"
    # layout per shard: q [128, TPC], k [128, TPC], v [128, NT, 130]
    with tile.TileContext(nc) as tc:
        with tc.tile_pool(name="dram", bufs=1, space="DRAM") as dram:
            a2a1_in = dram.tile([NCORES, 2 * 128 * TPC], F16)
            a2a1_out = dram.tile([NCORES, 2 * 128 * TPC], F16)
            a2av_in = dram.tile([NCORES, 128 * NT * 130], F16)
            a2av_out = dram.tile([NCORES, 128 * NT * 130], F16)
            a2a2_in = dram.tile([NCORES, 128 * NT * 2 * D], F16)
            a2a2_out = dram.tile([NCORES, 128 * NT * 2 * D], F16)

            _build_body(nc, tc, io, a2a1_in, a2a1_out, a2av_in, a2av_out,
                        a2a2_in, a2a2_out, skip_coll=skip_coll)
    nc.compile()
    return nc


def _build_body(nc, tc, io, a2a1_in, a2a1_out, a2av_in, a2av_out,
                a2a2_in, a2a2_out, skip_coll=False):
    from contextlib import ExitStack
    es = ExitStack()
    ident_pool = es.enter_context(tc.tile_pool(name="const", bufs=1))
    sb = es.enter_context(tc.tile_pool(name="sb", bufs=1))
    wl = es.enter_context(tc.tile_pool(name="wl", bufs=2))
    esp = ExitStack()
    ps = esp.enter_context(tc.tile_pool(name="mmps", bufs=3, space="PSUM"))
    psy = esp.enter_context(tc.tile_pool(name="psy", bufs=1, space="PSUM"))

    # ---------------- constants -------------------------------------------
    ident = ident_pool.tile([128, 128], F16)
    make_identity(nc, ident[:])
    t1 = ident_pool.tile([128, TPC], F16)
    t2 = ident_pool.tile([128, TPC], F16)
    nc.sync.dma_start(t1[:], io["ropeT1"][:])
    nc.sync.dma_start(t2[:], io["ropeT2"][:])
    jt = ident_pool.tile([128, 128], F16)
    nc.sync.dma_start(jt[:], io["ropeJT"][:])
    ones_col = ident_pool.tile([128, 1], F16)
    nc.gpsimd.memset(ones_col[:], 1.0)

    # ---------------- P0: x load + act_quant + transpose ------------------
    xsb = sb.tile([128, NT, C], F32)
    nc.sync.dma_start(xsb[:], io["x_slice"].rearrange("(n p) c -> p n c", p=128))
    xq16 = sb.tile([128, NT, C], F16)
    for tt in range(NT):
        mx = sb.tile([128, 1], F32, tag="mx")
        nc.vector.tensor_reduce(mx[:], xsb[:, tt], axis=AX.X, op=OP.max,
                                apply_absolute_value=True)
        sc = sb.tile([128, 1], F32, tag="sc")   # 1/st = clip(mx)/127
        nc.vector.tensor_scalar(sc[:], mx[:], 1e-5, 1.0 / 127.0,
                                op0=OP.max, op1=OP.mult)
        st = sb.tile([128, 1], F32, tag="st")   # 127/clip(mx)
        nc.vector.reciprocal(st[:], sc[:])
        xq8 = sb.tile([128, C], I8, tag="xq8")
        nc.vector.tensor_scalar(xq8[:], xsb[:, tt], st[:], None, op0=OP.mult)
        nc.vector.tensor_scalar(xq16[:, tt], xq8[:], sc[:], None, op0=OP.mult)
    # transpose -> xqT [c, t] tiles (PE transpose, psum bounce)
    xqT = sb.tile([128, NCT, TPC], F16)
    for ct in range(NCT):
        for tt in range(NT):
            trx = psy.tile([128, 128], F16, tag="trx")
            nc.tensor.transpose(trx[:], xq16[:, tt, 128 * ct:128 * (ct + 1)], ident[:])
            nc.scalar.activation(xqT[:, ct, 128 * tt:128 * (tt + 1)], trx[:], ACTF.Copy)

    # ---------------- weights helper ---------------------------------------
    wT = {}
    swcol = {}
    ones128 = ident_pool.tile([1, 128], F32)
    nc.gpsimd.memset(ones128[:], 1.0)
    onescol32 = ident_pool.tile([128, 1], F32)
    nc.gpsimd.memset(onescol32[:], 1.0)

    def prep_weight(wn):
        wsb = wl.tile([128, NCT, C], F32, tag="wload", name=f"wload_{wn}")
        nc.sync.dma_start(wsb[:], io[wn + "T"].rearrange("(n p) c -> p n c", p=128))
        asums = sb.tile([128, NCT], F32, tag="asums", name=f"asums_{wn}")
        for ot in range(NCT):
            nc.vector.tensor_reduce(asums[:, ot:ot + 1], wsb[:, ot], axis=AX.X,
                                    op=OP.add, apply_absolute_value=True)
        atot = sb.tile([128, 1], F32, tag="atot", name=f"atot_{wn}")
        nc.vector.tensor_reduce(atot[:], asums[:], axis=AX.X, op=OP.add)
        sw_ps = psy.tile([1, 1], F32, tag="swps", name=f"swps_{wn}")
        nc.tensor.matmul(sw_ps[:], onescol32[:], atot[:], start=True, stop=True)
        sw = sb.tile([1, 1], F32, tag="sw", name=f"sw_{wn}")
        nc.vector.tensor_scalar(sw[:], sw_ps[:], 1.0 / (C * C), 1e-5,
                                op0=OP.mult, op1=OP.max)
        swb_ps = psy.tile([128, 1], F32, tag="swbps", name=f"swbps_{wn}")
        nc.tensor.matmul(swb_ps[:], ones128[:], sw[:], start=True, stop=True)
        swc = sb.tile([128, 1], F32, tag=f"swc_{wn}", name=f"swc_{wn}")
        nc.vector.tensor_copy(swc[:], swb_ps[:])
        swcol[wn] = swc
        inv_s = sb.tile([128, 1], F32, tag="inv_s", name=f"invs_{wn}")
        nc.vector.reciprocal(inv_s[:], swc[:])
        wtag = "wTs_0" if wn in ("Wq", "Wv") else "wTs_1"
        wTt = sb.tile([128, NCT, C], F16, tag=wtag, name=f"wT_{wn}")
        for ot in range(NCT):
            w8 = sb.tile([128, C], I8, tag="w8", name=f"w8_{wn}{ot}", bufs=2)
            nc.vector.tensor_scalar(w8[:], wsb[:, ot], inv_s[:], None, op0=OP.mult)
            nc.vector.tensor_scalar(wTt[:, ot], w8[:], 1, -1,
                                    op0=OP.min, op1=OP.max)
        wT[wn] = wTt

    def proj_qk(name, dst):
        for ot in range(NCT):
            mm_ps = ps.tile([128, TPC], F32, tag="mm512", name=f"mmps_{name}{ot}")
            for ct in range(NCT):
                nc.tensor.matmul(mm_ps[:], wT[name][:, ct, 128 * ot:128 * (ot + 1)],
                                 xqT[:, ct], start=(ct == 0), stop=(ct == NCT - 1))
            raw = sb.tile([128, TPC], F16, tag="qkraw", name=f"raw_{name}{ot}")
            nc.vector.tensor_copy(raw[:], mm_ps[:])
            jq_ps = ps.tile([128, TPC], F32, tag="mm512", name=f"jq_{name}{ot}")
            nc.tensor.matmul(jq_ps[:], jt[:], raw[:], start=True, stop=True)
            p1 = sb.tile([128, TPC], F16, tag="ropep1", name=f"p1_{name}{ot}")
            p2 = sb.tile([128, TPC], F16, tag="ropep2", name=f"p2_{name}{ot}")
            nc.vector.tensor_tensor(p1[:], raw[:], t1[:], op=OP.mult)
            nc.vector.tensor_tensor(p2[:], jq_ps[:], t2[:], op=OP.mult)
            nc.vector.tensor_tensor(dst[:, ot], p1[:], p2[:], op=OP.add)

    QSZ = 128 * TPC
    VSZ = 128 * NT * 130

    # v pipeline -> atoa-v
    prep_weight("Wv")
    v_sb = sb.tile([128, NT, H, 65], F16)
    nc.gpsimd.memset(v_sb[:], 1.0)
    for tt in range(NT):
        for ob in range(2):
            mm_ps = ps.tile([128, 512], F32, tag="mm512", name=f"vps_{tt}{ob}")
            for ct in range(NCT):
                nc.tensor.matmul(mm_ps[:], xqT[:, ct, 128 * tt:128 * (tt + 1)],
                                 wT["Wv"][:, ct, 512 * ob:512 * (ob + 1)],
                                 start=(ct == 0), stop=(ct == NCT - 1))
            nc.scalar.activation(
                v_sb[:, tt, 8 * ob:8 * (ob + 1), 0:64], mm_ps[:],
                ACTF.Copy, scale=swcol["Wv"][:])
    for dst in range(NCORES):
        nc.sync.dma_start(
            a2av_in[dst].rearrange("(p n v) -> p n v", p=128, n=NT),
            v_sb[:, :, 2 * dst:2 * dst + 2, :].rearrange("p n h v -> p n (h v)"))
    if skip_coll:
        nc.sync.dma_start(a2av_out[:], a2av_in[:])
    else:
        nc.gpsimd.collective_compute(
            "AllToAll", OP.bypass, replica_groups=[list(range(NCORES))],
            ins=[a2av_in.opt()], outs=[a2av_out.opt()])

    va = sb.tile([128, BT // 128, 2, 65], F16, tag="v_sb")   # reuse v_sb slot
    for s in range(NCORES):
        nc.sync.dma_start(
            va[:, NT * s:NT * (s + 1)].rearrange("p n h v -> p n (h v)"),
            a2av_out[s].rearrange("(p n v) -> p n v", p=128, n=NT))

    # q/k pipeline -> atoa-qk as early as possible
    qTr = sb.tile([128, NCT, TPC], F16)
    kTr = sb.tile([128, NCT, TPC], F16)
    prep_weight("Wq")
    proj_qk("Wq", qTr)
    nc.sync.dma_start(a2a1_in[:, 0:QSZ].rearrange("d (p t) -> p d t", p=128), qTr[:])
    prep_weight("Wk")
    proj_qk("Wk", kTr)
    nc.sync.dma_start(a2a1_in[:, QSZ:2 * QSZ].rearrange("d (p t) -> p d t", p=128),
                      kTr[:])
    if skip_coll:
        nc.sync.dma_start(a2a1_out[:], a2a1_in[:])
    else:
        nc.gpsimd.collective_compute(
            "AllToAll", OP.bypass, replica_groups=[list(range(NCORES))],
            ins=[a2a1_in.opt()], outs=[a2a1_out.opt()])

    qTa = sb.tile([128, BT], F16, tag="qTr")     # reuse qTr slot (dead after send)
    kTa = sb.tile([128, BT], F16, tag="kTr")
    nc.scalar.dma_start(qTa[:].rearrange("p (s t) -> p s t", s=NCORES),
                        a2a1_out[:, 0:QSZ].rearrange("s (p t) -> p s t", p=128))
    nc.scalar.dma_start(kTa[:].rearrange("p (s t) -> p s t", s=NCORES),
                        a2a1_out[:, QSZ:2 * QSZ].rearrange("s (p t) -> p s t", p=128))

    # exp scale column: swq*swk/8 -> [128,1] f32
    expsc = sb.tile([128, 1], F32)
    nc.vector.tensor_tensor(expsc[:], swcol["Wq"][:], swcol["Wk"][:], op=OP.mult)
    nc.vector.tensor_scalar(expsc[:], expsc[:], 1.0 / np.sqrt(np.float64(D)), None,
                            op0=OP.mult)

    # Wo prep overlaps the collectives / attention start
    prep_weight("Wo")

    # ---------------- P4: attention ----------------------------------------
    # per (head, batch, q-block): scores (transposed) -> exp -> mask -> AV
    esp.close()
    y_sb = sb.tile([128, BT // 128, 2, D], F16)   # [qt-part, qt-tile, head, d]
    exp_sb_pool = es.enter_context(tc.tile_pool(name="expp", bufs=3))
    esp = ExitStack()
    score_ps_pool = esp.enter_context(tc.tile_pool(name="scps", bufs=2, space="PSUM"))
    yaug_ps_pool = esp.enter_context(tc.tile_pool(name="yaug", bufs=2, space="PSUM"))
    tr_ps_pool = esp.enter_context(tc.tile_pool(name="trps", bufs=2, space="PSUM"))
    KG = 2          # k-tiles per exp group (psum banks per score group)
    for b in range(B):
        base = b * T
        for jb in range(NQB):
            qs = base + QB * jb           # q-block col offset
            for h in range(HPC):
                yaug = yaug_ps_pool.tile([65, QB], F32, tag="yaug")
                nkt = (jb + 1) * (QB // KT)       # causal k-tiles for this block
                for kg in range(nkt // KG):
                    sgrp = score_ps_pool.tile([128, KG * QB], F32, tag="sgrp")
                    for m in range(KG):
                        kt_i = kg * KG + m
                        ks = base + KT * kt_i
                        nc.tensor.matmul(
                            sgrp[:, QB * m:QB * (m + 1)],
                            kTa[64 * h:64 * (h + 1), ks:ks + KT],
                            qTa[64 * h:64 * (h + 1), qs:qs + QB],
                            start=True, stop=True,
                            tile_position=(64 * h, 0))
                    egrp = exp_sb_pool.tile([128, KG * QB], F16, tag=f"egrp{h}")
                    nc.scalar.activation(egrp[:], sgrp[:], ACTF.Exp, scale=expsc[:])
                    for m in range(KG):
                        kt_i = kg * KG + m
                        mbase = QB * jb - KT * kt_i
                        if mbase < 127:   # diagonal tile: causal mask needed
                            nc.gpsimd.affine_select(
                                out=egrp[:, QB * m:QB * (m + 1)],
                                in_=egrp[:, QB * m:QB * (m + 1)],
                                compare_op=OP.is_ge, fill=0.0,
                                base=mbase, pattern=[[1, QB]],
                                channel_multiplier=-1)
                    for m in range(KG):
                        kt_i = kg * KG + m
                        gt = base // 128 + kt_i
                        nc.tensor.matmul(yaug[:], va[:, gt, h, :],
                                         egrp[:, QB * m:QB * (m + 1)],
                                         start=(kg == 0 and m == 0),
                                         stop=(kg == nkt // KG - 1 and m == KG - 1))
                # epilogue: copy, transpose 128-chunks, normalize
                yaug16 = exp_sb_pool.tile([65, QB], F16, tag=f"yaug16_{h}")
                nc.vector.tensor_copy(yaug16[:], yaug[:])
                for ch in range(QB // 128):
                    trp = tr_ps_pool.tile([128, 65], F16, tag="trp")
                    nc.tensor.transpose(trp[:], yaug16[:, 128 * ch:128 * (ch + 1)],
                                        ident[0:65, 0:65])
                    rec = exp_sb_pool.tile([128, 1], F32, tag=f"rec{h}")
                    nc.vector.reciprocal(rec[:], trp[:, 64:65])
                    nc.vector.tensor_scalar(
                        y_sb[:, (qs + 128 * ch) // 128, h, :], trp[:, 0:64],
                        rec[:], None, op0=OP.mult)

    # ---------------- P5: AllToAll #2 --------------------------------------
    YSZ = 128 * NT * 2 * D
    for dst in range(NCORES):
        nc.sync.dma_start(
            a2a2_in[dst].rearrange("(p n c) -> p n c", p=128, n=NT),
            y_sb[:, NT * dst:NT * (dst + 1)].rearrange("p n h dd -> p n (h dd)"))
    if skip_coll:
        nc.sync.dma_start(a2a2_out[:], a2a2_in[:])
    else:
        nc.gpsimd.collective_compute(
            "AllToAll", OP.bypass, replica_groups=[list(range(NCORES))],
            ins=[a2a2_in.opt()], outs=[a2a2_out.opt()])
    yfull = sb.tile([128, NT, C], F16, tag="xq16")   # [t-part, t-tile, channels]
    for s in range(NCORES):
        nc.sync.dma_start(
            yfull[:, :, 128 * s:128 * (s + 1)],
            a2a2_out[s].rearrange("(p n c) -> p n c", p=128, n=NT))

    # act_quant(y) exact int8 + transpose
    esp.close()
    esp = ExitStack()
    ps = esp.enter_context(tc.tile_pool(name="ops", bufs=4, space="PSUM"))
    yq16 = sb.tile([128, NT, C], F16)
    osc = {}
    for tt in range(NT):
        mxy = sb.tile([128, 1], F32, tag="mxy")
        nc.vector.tensor_reduce(mxy[:], yfull[:, tt], axis=AX.X, op=OP.max,
                                apply_absolute_value=True)
        scy = sb.tile([128, 1], F32, tag=f"scy{tt}")
        nc.vector.tensor_scalar(scy[:], mxy[:], 1e-5, 1.0 / 127.0,
                                op0=OP.max, op1=OP.mult)
        sty = sb.tile([128, 1], F32, tag="sty")
        nc.vector.reciprocal(sty[:], scy[:])
        yq8 = sb.tile([128, C], I8, tag="yq8")
        nc.vector.tensor_scalar(yq8[:], yfull[:, tt], sty[:], None, op0=OP.mult)
        nc.vector.tensor_copy(yq16[:, tt], yq8[:])
        # output scale column: swo * scy
        oscc = sb.tile([128, 1], F32, tag=f"oscc{tt}")
        nc.vector.tensor_tensor(oscc[:], scy[:], swcol["Wo"][:], op=OP.mult)
        osc[tt] = oscc
    yqT = sb.tile([128, NCT, TPC], F16)
    for ct in range(NCT):
        for tt in range(NT):
            trx = ps.tile([128, 128], F16, tag="trx")
            nc.tensor.transpose(trx[:], yq16[:, tt, 128 * ct:128 * (ct + 1)], ident[:])
            nc.vector.tensor_copy(yqT[:, ct, 128 * tt:128 * (tt + 1)], trx[:])

    # ---------------- P6: Wo projection ------------------------------------
    out_sb = sb.tile([128, NT, C], F32, tag="xsb")
    for tt in range(NT):
        for ob in range(2):
            mm_ps = ps.tile([128, 512], F32, tag="mm512")
            for ct in range(NCT):
                nc.tensor.matmul(mm_ps[:], yqT[:, ct, 128 * tt:128 * (tt + 1)],
                                 wT["Wo"][:, ct, 512 * ob:512 * (ob + 1)],
                                 start=(ct == 0), stop=(ct == NCT - 1))
            nc.scalar.activation(out_sb[:, tt, 512 * ob:512 * (ob + 1)], mm_ps[:],
                                 ACTF.Copy, scale=osc[tt][:])
            nc.sync.dma_start(
                io["out_slice"].rearrange("(n p) c -> p n c", p=128)
                [:, tt, 512 * ob:512 * (ob + 1)],
                out_sb[:, tt, 512 * ob:512 * (ob + 1)])
    esp.close()
    es.close()


def kernel(x, Wq, Wk, Wv, Wo, _trace=False):
    x = np.ascontiguousarray(x, dtype=np.float32)
    if "nc" not in _CACHE:
        _CACHE["nc"] = build_program()
    nc = _CACHE["nc"]
    xf = x.reshape(BT, C)
    wqT = np.ascontiguousarray(np.asarray(Wq, np.float32).T)
    wkT = np.ascontiguousarray(np.asarray(Wk, np.float32).T)
    wvT = np.ascontiguousarray(np.asarray(Wv, np.float32).T)
    woT = np.ascontiguousarray(np.asarray(Wo, np.float32).T)
    in_maps = []
    for c in range(NCORES):
        t1, t2 = _host_tables(c)
        in_maps.append({
            "x_slice": np.ascontiguousarray(xf[TPC * c:TPC * (c + 1)]),
            "WqT": wqT, "WkT": wkT, "WvT": wvT, "WoT": woT,
            "ropeT1": t1, "ropeT2": t2, "ropeJT": _host_jt(),
        })
    res = run_bass_kernel_spmd(nc, in_maps, list(range(NCORES)), trace=_trace)
    out = np.concatenate([res.results[c]["out_slice"] for c in range(NCORES)], axis=0)
    out = out.reshape(B, T, C).astype(np.float32)
    if _trace:
        return out, res
    return out


# revision 32
# speedup vs baseline: 1.0487x; 1.0035x over previous
"""Trainium2 Bass kernel for nn_CausalSelfAttention_52905407152466.

BitNet-style causal self-attention, distributed over 8 NeuronCores:
  - token-sharded QKV projections (512 tokens/core, full weights/core)
  - AllToAll #1 -> head-sharded attention (2 heads x B=2 per core)
  - AllToAll #2 -> token-sharded output projection

Numeric strategy: ternary weights are exact in fp16, so all projection
matmuls run in fp16 losslessly given fp16 activations. Attention runs in
fp16 (inputs ~2^-11 rounded, fp32 accumulation). The softmax skips the
max-subtraction (scores are bounded ~4) so exp folds into one activation
instruction per score group; the normalizer Z comes from an ones-column
appended to V. Causal masking = gpsimd affine_select on the exp output of
diagonal tiles. The final Wo projection uses the exact int8 path
(int8 x ternary in fp16 = exact integer accumulation in fp32).
"""

import numpy as np

import concourse.bacc as bacc
import concourse.mybir as mybir
import concourse.tile as tile
from concourse.bass_utils import run_bass_kernel_spmd
from concourse.masks import make_identity

F32 = mybir.dt.float32
F16 = mybir.dt.float16
I8 = mybir.dt.int8
AX = mybir.AxisListType
OP = mybir.AluOpType
ACTF = mybir.ActivationFunctionType

NCORES = 8
B, T, C = 2, 2048, 1024
H, D = 16, 64
BT = B * T                  # 4096 flat tokens
TPC = BT // NCORES          # 512 tokens per core
HPC = H // NCORES           # 2 heads per core
NT = TPC // 128             # 4 token tiles per core
NCT = C // 128              # 8 channel tiles
QB = 512                    # query block (free dim of score matmuls)
NQB = T // QB               # 4 query blocks per (b, h) instance
KT = 128                    # key tile (partition dim of scores)
ROPE_BASE = 10000.0

_CACHE = {}


def _host_tables(core):
    """Per-core RoPE tables in [128 = 2 interleaved heads x (32 lo | 32 hi), TPC] f16."""
    pos0 = (core * TPC) % T
    pos = np.arange(pos0, pos0 + TPC, dtype=np.float64)
    inv = 1.0 / (ROPE_BASE ** (np.arange(0, D, 2, dtype=np.float64) / D))
    ang = pos[None, :] * inv[:, None]              # [32, TPC]
    cos = np.cos(ang).astype(np.float32).astype(np.float16)
    sin = np.sin(ang).astype(np.float32).astype(np.float16)
    # rope as q*cos + (J q)*sin with J the half-swap sign matrix
    t1 = np.concatenate([cos, cos, cos, cos], axis=0)
    t2 = np.concatenate([sin, sin, sin, sin], axis=0)
    return t1.astype(np.float16), t2.astype(np.float16)


def _host_jt():
    i32 = np.eye(32, dtype=np.float16)
    z = np.zeros((32, 32), np.float16)
    j64 = np.block([[z, -i32], [i32, z]])     # J: Jq[0:32] = -q[32:64]; Jq[32:64] = q[0:32]
    jt = np.block([[j64.T, np.zeros((64, 64), np.float16)],
                   [np.zeros((64, 64), np.float16), j64.T]])
    return jt.astype(np.float16)


def build_program():
    nc = bacc.Bacc("TRN2", target_bir_lowering=False, debug=False,
                   num_devices=NCORES)
    io = {}

    def inp(name, shape, dtype=F32):
        io[name] = nc.declare_dram_parameter(name, list(shape), dtype, isOutput=False)
        return io[name]

    def outp(name, shape, dtype=F32):
        io[name] = nc.declare_dram_parameter(name, list(shape), dtype, isOutput=True)
        return io[name]

    x_d = inp("x_slice", (TPC, C))
    w_d = {n: inp(n + "T", (C, C)) for n in ("Wq", "Wk", "Wv", "Wo")}
    t1_d = inp("ropeT1", (128, TPC), F16)
    t2_d = inp("ropeT2", (128, TPC), F16)
    jt_d = inp("ropeJT", (128, 128), F16)
    out_d = outp("out_slice", (TPC, C))

    import os
    skip_coll = os.environ.get("SKIP_COLL", "0") == "1"
    # layout per shard: q [128, TPC], k [128, TPC], v [128, NT, 130]
    with tile.TileContext(nc) as tc:
        with tc.tile_pool(name="dram", bufs=1, space="DRAM") as dram:
            a2a1_in = dram.tile([NCORES, 2 * 128 * TPC], F16)
            a2a1_out = dram.tile([NCORES, 2 * 128 * TPC], F16)
            a2av_in = dram.tile([NCORES, 128 * NT * 130], F16)
            a2av_out = dram.tile([NCORES, 128 * NT * 130], F16)
            a2a2_in = dram.tile([NCORES, 128 * NT * 2 * D], F16)
            a2a2_out = dram.tile([NCORES, 128 * NT * 2 * D], F16)

            _build_body(nc, tc, io, a2a1_in, a2a1_out, a2av_in, a2av_out,
                        a2a2_in, a2a2_out, skip_coll=skip_coll)
    nc.compile()
    return nc


def _build_body(nc, tc, io, a2a1_in, a2a1_out, a2av_in, a2av_out,
                a2a2_in, a2a2_out, skip_coll=False):
    from contextlib import ExitStack
    es = ExitStack()
    ident_pool = es.enter_context(tc.tile_pool(name="const", bufs=1))
    sb = es.enter_context(tc.tile_pool(name="sb", bufs=1))
    wl = es.enter_context(tc.tile_pool(name="wl", bufs=2))
    esp = ExitStack()
    ps = esp.enter_context(tc.tile_pool(name="mmps", bufs=3, space="PSUM"))
    psy = esp.enter_context(tc.tile_pool(name="psy", bufs=1, space="PSUM"))

    # ---------------- constants -------------------------------------------
    ident = ident_pool.tile([128, 128], F16)
    make_identity(nc, ident[:])
    t1 = ident_pool.tile([128, TPC], F16)
    t2 = ident_pool.tile([128, TPC], F16)
    nc.sync.dma_start(t1[:], io["ropeT1"][:])
    nc.sync.dma_start(t2[:], io["ropeT2"][:])
    jt = ident_pool.tile([128, 128], F16)
    nc.sync.dma_start(jt[:], io["ropeJT"][:])
    ones_col = ident_pool.tile([128, 1], F16)
    nc.gpsimd.memset(ones_col[:], 1.0)

    # ---------------- P0: x load + act_quant + transpose ------------------
    xsb = sb.tile([128, NT, C], F32)
    nc.sync.dma_start(xsb[:], io["x_slice"].rearrange("(n p) c -> p n c", p=128))
    xq16 = sb.tile([128, NT, C], F16)
    for tt in range(NT):
        mx = sb.tile([128, 1], F32, tag="mx")
        nc.vector.tensor_reduce(mx[:], xsb[:, tt], axis=AX.X, op=OP.max,
                                apply_absolute_value=True)
        sc = sb.tile([128, 1], F32, tag="sc")   # 1/st = clip(mx)/127
        nc.vector.tensor_scalar(sc[:], mx[:], 1e-5, 1.0 / 127.0,
                                op0=OP.max, op1=OP.mult)
        st = sb.tile([128, 1], F32, tag="st")   # 127/clip(mx)
        nc.vector.reciprocal(st[:], sc[:])
        xq8 = sb.tile([128, C], I8, tag="xq8")
        nc.vector.tensor_scalar(xq8[:], xsb[:, tt], st[:], None, op0=OP.mult)
        nc.vector.tensor_scalar(xq16[:, tt], xq8[:], sc[:], None, op0=OP.mult)
    # transpose -> xqT [c, t] tiles (PE transpose, psum bounce)
    xqT = sb.tile([128, NCT, TPC], F16)
    for ct in range(NCT):
        for tt in range(NT):
            trx = psy.tile([128, 128], F16, tag="trx")
            nc.tensor.transpose(trx[:], xq16[:, tt, 128 * ct:128 * (ct + 1)], ident[:])
            nc.scalar.activation(xqT[:, ct, 128 * tt:128 * (tt + 1)], trx[:], ACTF.Copy)

    # ---------------- weights helper ---------------------------------------
    wT = {}
    swcol = {}
    ones128 = ident_pool.tile([1, 128], F32)
    nc.gpsimd.memset(ones128[:], 1.0)
    onescol32 = ident_pool.tile([128, 1], F32)
    nc.gpsimd.memset(onescol32[:], 1.0)

    def prep_weight(wn, tern_eng=None):
        tern_eng = tern_eng or nc.vector
        wsb = wl.tile([128, NCT, C], F32, tag="wload", name=f"wload_{wn}")
        nc.sync.dma_start(wsb[:], io[wn + "T"].rearrange("(n p) c -> p n c", p=128))
        asums = sb.tile([128, NCT], F32, tag="asums", name=f"asums_{wn}")
        for ot in range(NCT):
            nc.vector.tensor_reduce(asums[:, ot:ot + 1], wsb[:, ot], axis=AX.X,
                                    op=OP.add, apply_absolute_value=True)
        atot = sb.tile([128, 1], F32, tag="atot", name=f"atot_{wn}")
        nc.vector.tensor_reduce(atot[:], asums[:], axis=AX.X, op=OP.add)
        sw_ps = psy.tile([1, 1], F32, tag="swps", name=f"swps_{wn}")
        nc.tensor.matmul(sw_ps[:], onescol32[:], atot[:], start=True, stop=True)
        sw = sb.tile([1, 1], F32, tag="sw", name=f"sw_{wn}")
        nc.vector.tensor_scalar(sw[:], sw_ps[:], 1.0 / (C * C), 1e-5,
                                op0=OP.mult, op1=OP.max)
        swb_ps = psy.tile([128, 1], F32, tag="swbps", name=f"swbps_{wn}")
        nc.tensor.matmul(swb_ps[:], ones128[:], sw[:], start=True, stop=True)
        swc = sb.tile([128, 1], F32, tag=f"swc_{wn}", name=f"swc_{wn}")
        nc.vector.tensor_copy(swc[:], swb_ps[:])
        swcol[wn] = swc
        inv_s = sb.tile([128, 1], F32, tag="inv_s", name=f"invs_{wn}")
        nc.vector.reciprocal(inv_s[:], swc[:])
        wtag = "wTs_0" if wn in ("Wq", "Wv") else "wTs_1"
        wTt = sb.tile([128, NCT, C], F16, tag=wtag, name=f"wT_{wn}")
        for ot in range(NCT):
            w8 = sb.tile([128, C], I8, tag="w8", name=f"w8_{wn}{ot}", bufs=2)
            tern_eng.tensor_scalar(w8[:], wsb[:, ot], inv_s[:], None, op0=OP.mult)
            tern_eng.tensor_scalar(wTt[:, ot], w8[:], 1, -1,
                                   op0=OP.min, op1=OP.max)
        wT[wn] = wTt

    def proj_qk(name, dst):
        for ot in range(NCT):
            mm_ps = ps.tile([128, TPC], F32, tag="mm512", name=f"mmps_{name}{ot}")
            for ct in range(NCT):
                nc.tensor.matmul(mm_ps[:], wT[name][:, ct, 128 * ot:128 * (ot + 1)],
                                 xqT[:, ct], start=(ct == 0), stop=(ct == NCT - 1))
            raw = sb.tile([128, TPC], F16, tag="qkraw", name=f"raw_{name}{ot}")
            nc.vector.tensor_copy(raw[:], mm_ps[:])
            jq_ps = ps.tile([128, TPC], F32, tag="mm512", name=f"jq_{name}{ot}")
            nc.tensor.matmul(jq_ps[:], jt[:], raw[:], start=True, stop=True)
            p1 = sb.tile([128, TPC], F16, tag="ropep1", name=f"p1_{name}{ot}")
            p2 = sb.tile([128, TPC], F16, tag="ropep2", name=f"p2_{name}{ot}")
            nc.vector.tensor_tensor(p1[:], raw[:], t1[:], op=OP.mult)
            nc.vector.tensor_tensor(p2[:], jq_ps[:], t2[:], op=OP.mult)
            nc.vector.tensor_tensor(dst[:, ot], p1[:], p2[:], op=OP.add)

    QSZ = 128 * TPC
    VSZ = 128 * NT * 130

    # v pipeline -> atoa-v
    prep_weight("Wv")
    v_sb = sb.tile([128, NT, H, 65], F16)
    nc.gpsimd.memset(v_sb[:], 1.0)
    for tt in range(NT):
        for ob in range(2):
            mm_ps = ps.tile([128, 512], F32, tag="mm512", name=f"vps_{tt}{ob}")
            for ct in range(NCT):
                nc.tensor.matmul(mm_ps[:], xqT[:, ct, 128 * tt:128 * (tt + 1)],
                                 wT["Wv"][:, ct, 512 * ob:512 * (ob + 1)],
                                 start=(ct == 0), stop=(ct == NCT - 1))
            nc.scalar.activation(
                v_sb[:, tt, 8 * ob:8 * (ob + 1), 0:64], mm_ps[:],
                ACTF.Copy, scale=swcol["Wv"][:])
    for dst in range(NCORES):
        nc.sync.dma_start(
            a2av_in[dst].rearrange("(p n v) -> p n v", p=128, n=NT),
            v_sb[:, :, 2 * dst:2 * dst + 2, :].rearrange("p n h v -> p n (h v)"))
    if skip_coll:
        nc.sync.dma_start(a2av_out[:], a2av_in[:])
    else:
        nc.gpsimd.collective_compute(
            "AllToAll", OP.bypass, replica_groups=[list(range(NCORES))],
            ins=[a2av_in.opt()], outs=[a2av_out.opt()])

    va = sb.tile([128, BT // 128, 2, 65], F16, tag="v_sb")   # reuse v_sb slot
    for s in range(NCORES):
        nc.sync.dma_start(
            va[:, NT * s:NT * (s + 1)].rearrange("p n h v -> p n (h v)"),
            a2av_out[s].rearrange("(p n v) -> p n v", p=128, n=NT))

    # q/k pipeline -> atoa-qk as early as possible
    qTr = sb.tile([128, NCT, TPC], F16)
    kTr = sb.tile([128, NCT, TPC], F16)
    prep_weight("Wq")
    proj_qk("Wq", qTr)
    nc.sync.dma_start(a2a1_in[:, 0:QSZ].rearrange("d (p t) -> p d t", p=128), qTr[:])
    prep_weight("Wk")
    proj_qk("Wk", kTr)
    nc.sync.dma_start(a2a1_in[:, QSZ:2 * QSZ].rearrange("d (p t) -> p d t", p=128),
                      kTr[:])
    if skip_coll:
        nc.sync.dma_start(a2a1_out[:], a2a1_in[:])
    else:
        nc.gpsimd.collective_compute(
            "AllToAll", OP.bypass, replica_groups=[list(range(NCORES))],
            ins=[a2a1_in.opt()], outs=[a2a1_out.opt()])

    qTa = sb.tile([128, BT], F16, tag="qTr")     # reuse qTr slot (dead after send)
    kTa = sb.tile([128, BT], F16, tag="kTr")
    nc.scalar.dma_start(qTa[:].rearrange("p (s t) -> p s t", s=NCORES),
                        a2a1_out[:, 0:QSZ].rearrange("s (p t) -> p s t", p=128))
    nc.scalar.dma_start(kTa[:].rearrange("p (s t) -> p s t", s=NCORES),
                        a2a1_out[:, QSZ:2 * QSZ].rearrange("s (p t) -> p s t", p=128))

    # exp scale column: swq*swk/8 -> [128,1] f32
    expsc = sb.tile([128, 1], F32)
    nc.vector.tensor_tensor(expsc[:], swcol["Wq"][:], swcol["Wk"][:], op=OP.mult)
    nc.vector.tensor_scalar(expsc[:], expsc[:], 1.0 / np.sqrt(np.float64(D)), None,
                            op0=OP.mult)

    # Wo prep overlaps the collectives / attention start
    prep_weight("Wo", tern_eng=nc.gpsimd)

    # ---------------- P4: attention ----------------------------------------
    # per (head, batch, q-block): scores (transposed) -> exp -> mask -> AV
    esp.close()
    y_sb = sb.tile([128, BT // 128, 2, D], F16)   # [qt-part, qt-tile, head, d]
    exp_sb_pool = es.enter_context(tc.tile_pool(name="expp", bufs=3))
    esp = ExitStack()
    score_ps_pool = esp.enter_context(tc.tile_pool(name="scps", bufs=2, space="PSUM"))
    yaug_ps_pool = esp.enter_context(tc.tile_pool(name="yaug", bufs=2, space="PSUM"))
    tr_ps_pool = esp.enter_context(tc.tile_pool(name="trps", bufs=2, space="PSUM"))
    KG = 2          # k-tiles per exp group (psum banks per score group)
    for b in range(B):
        base = b * T
        for jb in range(NQB):
            qs = base + QB * jb           # q-block col offset
            for h in range(HPC):
                yaug = yaug_ps_pool.tile([65, QB], F32, tag="yaug")
                nkt = (jb + 1) * (QB // KT)       # causal k-tiles for this block
                for kg in range(nkt // KG):
                    sgrp = score_ps_pool.tile([128, KG * QB], F32, tag="sgrp")
                    for m in range(KG):
                        kt_i = kg * KG + m
                        ks = base + KT * kt_i
                        nc.tensor.matmul(
                            sgrp[:, QB * m:QB * (m + 1)],
                            kTa[64 * h:64 * (h + 1), ks:ks + KT],
                            qTa[64 * h:64 * (h + 1), qs:qs + QB],
                            start=True, stop=True,
                            tile_position=(64 * h, 0))
                    egrp = exp_sb_pool.tile([128, KG * QB], F16, tag=f"egrp{h}")
                    nc.scalar.activation(egrp[:], sgrp[:], ACTF.Exp, scale=expsc[:])
                    for m in range(KG):
                        kt_i = kg * KG + m
                        mbase = QB * jb - KT * kt_i
                        if mbase < 127:   # diagonal tile: causal mask needed
                            nc.gpsimd.affine_select(
                                out=egrp[:, QB * m:QB * (m + 1)],
                                in_=egrp[:, QB * m:QB * (m + 1)],
                                compare_op=OP.is_ge, fill=0.0,
                                base=mbase, pattern=[[1, QB]],
                                channel_multiplier=-1)
                    for m in range(KG):
                        kt_i = kg * KG + m
                        gt = base // 128 + kt_i
                        nc.tensor.matmul(yaug[:], va[:, gt, h, :],
                                         egrp[:, QB * m:QB * (m + 1)],
                                         start=(kg == 0 and m == 0),
                                         stop=(kg == nkt // KG - 1 and m == KG - 1))
                # epilogue: copy, transpose 128-chunks, normalize
                yaug16 = exp_sb_pool.tile([65, QB], F16, tag=f"yaug16_{h}")
                nc.vector.tensor_copy(yaug16[:], yaug[:])
                for ch in range(QB // 128):
                    trp = tr_ps_pool.tile([128, 65], F16, tag="trp")
                    nc.tensor.transpose(trp[:], yaug16[:, 128 * ch:128 * (ch + 1)],
                                        ident[0:65, 0:65])
                    rec = exp_sb_pool.tile([128, 1], F32, tag=f"rec{h}")
                    nc.vector.reciprocal(rec[:], trp[:, 64:65])
                    nc.vector.tensor_scalar(
                        y_sb[:, (qs + 128 * ch) // 128, h, :], trp[:, 0:64],
                        rec[:], None, op0=OP.mult)

    # ---------------- P5: AllToAll #2 --------------------------------------
    YSZ = 128 * NT * 2 * D
    for dst in range(NCORES):
        nc.sync.dma_start(
            a2a2_in[dst].rearrange("(p n c) -> p n c", p=128, n=NT),
            y_sb[:, NT * dst:NT * (dst + 1)].rearrange("p n h dd -> p n (h dd)"))
    if skip_coll:
        nc.sync.dma_start(a2a2_out[:], a2a2_in[:])
    else:
        nc.gpsimd.collective_compute(
            "AllToAll", OP.bypass, replica_groups=[list(range(NCORES))],
            ins=[a2a2_in.opt()], outs=[a2a2_out.opt()])
    yfull = sb.tile([128, NT, C], F16, tag="xq16")   # [t-part, t-tile, channels]
    for s in range(NCORES):
        nc.sync.dma_start(
            yfull[:, :, 128 * s:128 * (s + 1)],
            a2a2_out[s].rearrange("(p n c) -> p n c", p=128, n=NT))

    # act_quant(y) exact int8 + transpose
    esp.close()
    esp = ExitStack()
    ps = esp.enter_context(tc.tile_pool(name="ops", bufs=4, space="PSUM"))
    yq16 = sb.tile([128, NT, C], F16)
    osc = {}
    for tt in range(NT):
        mxy = sb.tile([128, 1], F32, tag="mxy")
        nc.vector.tensor_reduce(mxy[:], yfull[:, tt], axis=AX.X, op=OP.max,
                                apply_absolute_value=True)
        scy = sb.tile([128, 1], F32, tag=f"scy{tt}")
        nc.vector.tensor_scalar(scy[:], mxy[:], 1e-5, 1.0 / 127.0,
                                op0=OP.max, op1=OP.mult)
        sty = sb.tile([128, 1], F32, tag="sty")
        nc.vector.reciprocal(sty[:], scy[:])
        yq8 = sb.tile([128, C], I8, tag="yq8")
        nc.vector.tensor_scalar(yq8[:], yfull[:, tt], sty[:], None, op0=OP.mult)
        nc.vector.tensor_copy(yq16[:, tt], yq8[:])
        # output scale column: swo * scy
        oscc = sb.tile([128, 1], F32, tag=f"oscc{tt}")
        nc.vector.tensor_tensor(oscc[:], scy[:], swcol["Wo"][:], op=OP.mult)
        osc[tt] = oscc
    yqT = sb.tile([128, NCT, TPC], F16)
    for ct in range(NCT):
        for tt in range(NT):
            trx = ps.tile([128, 128], F16, tag="trx")
            nc.tensor.transpose(trx[:], yq16[:, tt, 128 * ct:128 * (ct + 1)], ident[:])
            nc.vector.tensor_copy(yqT[:, ct, 128 * tt:128 * (tt + 1)], trx[:])

    # ---------------- P6: Wo projection ------------------------------------
    out_sb = sb.tile([128, NT, C], F32, tag="xsb")
    for tt in range(NT):
        for ob in range(2):
            mm_ps = ps.tile([128, 512], F32, tag="mm512")
            for ct in range(NCT):
                nc.tensor.matmul(mm_ps[:], yqT[:, ct, 128 * tt:128 * (tt + 1)],
                                 wT["Wo"][:, ct, 512 * ob:512 * (ob + 1)],
                                 start=(ct == 0), stop=(ct == NCT - 1))
            nc.scalar.activation(out_sb[:, tt, 512 * ob:512 * (ob + 1)], mm_ps[:],
                                 ACTF.Copy, scale=osc[tt][:])
            nc.sync.dma_start(
                io["out_slice"].rearrange("(n p) c -> p n c", p=128)
                [:, tt, 512 * ob:512 * (ob + 1)],
                out_sb[:, tt, 512 * ob:512 * (ob + 1)])
    esp.close()
    es.close()


def kernel(x, Wq, Wk, Wv, Wo, _trace=False):
    x = np.ascontiguousarray(x, dtype=np.float32)
    if "nc" not in _CACHE:
        _CACHE["nc"] = build_program()
    nc = _CACHE["nc"]
    xf = x.reshape(BT, C)
    wqT = np.ascontiguousarray(np.asarray(Wq, np.float32).T)
    wkT = np.ascontiguousarray(np.asarray(Wk, np.float32).T)
    wvT = np.ascontiguousarray(np.asarray(Wv, np.float32).T)
    woT = np.ascontiguousarray(np.asarray(Wo, np.float32).T)
    in_maps = []
    for c in range(NCORES):
        t1, t2 = _host_tables(c)
        in_maps.append({
            "x_slice": np.ascontiguousarray(xf[TPC * c:TPC * (c + 1)]),
            "WqT": wqT, "WkT": wkT, "WvT": wvT, "WoT": woT,
            "ropeT1": t1, "ropeT2": t2, "ropeJT": _host_jt(),
        })
    res = run_bass_kernel_spmd(nc, in_maps, list(range(NCORES)), trace=_trace)
    out = np.concatenate([res.results[c]["out_slice"] for c in range(NCORES)], axis=0)
    out = out.reshape(B, T, C).astype(np.float32)
    if _trace:
        return out, res
    return out
